# revision 5
# baseline (speedup 1.0000x reference)
"""Distributed Trainium2 kernel for GQA attention block (dense_transformer).

Sharding: DP2 over batch x TP4 over heads (8 cores).
  core c -> batch b = c//4, head-group g = c%4 (8 q heads, 2 kv heads).
Per-core pipeline (one SPMD graph, bf16 matmuls, f32 accumulation):
  phase A: transposed QKV projections (qT/kT: [hd, seq] layout) + fused RoPE
  phase B: scores^T = K_r^T-chunk @ q_r (per head), masked exp via ACT,
           attnV + replicated-ones rowsum matmuls, normalize epilogue
  AllGather (4-core groups) of per-head attention outputs (bf16, blocked)
  phase C: out slice = attn_full @ wo[:, g*1024:(g+1)*1024]
Causal (tril) mask specialization: fully-masked score blocks are skipped;
their exp(0)=1 contributions are added analytically (suffix V sums + counts).
"""

import sys

sys.path.insert(0, "/opt/trn_rl_repo")

import numpy as np
import ml_dtypes

import concourse.bass as bass
import concourse.mybir as mybir
import concourse.tile as tile
from concourse import bacc
from concourse.bass_utils import run_bass_kernel_spmd

BF16 = mybir.dt.bfloat16
F32 = mybir.dt.float32

B, S, D = 2, 2048, 4096
H, KVH, HD = 32, 8, 128
HALF = HD // 2
N_CORES = 8
TPG = 4  # tensor-parallel group size
HPC = H // TPG  # 8 q heads / core
KVPC = KVH // TPG  # 2 kv heads / core
NSLICE = D // TPG  # 1024 output columns / core
SC = 512  # seq chunk (phase A)
NSC = S // SC  # 4
KO = D // 128  # 32 contraction chunks
QB = 512  # q chunk (phase B)
NQB = S // QB  # 4
KB = 128  # k chunk (phase B)
NKB = S // KB  # 16
SCALE = 1.0 / float(np.sqrt(HD))
GROUPS = [[0, 1, 2, 3], [4, 5, 6, 7]]

LAST_EXEC_NS = None
_CACHE = {}


def _build(mask_mode):
    """mask_mode: 'tril' (causal specialization), 'ones' (no mask), 'full'."""
    nc = bacc.Bacc("TRN2", target_bir_lowering=False, debug=False, num_devices=N_CORES)

    xT = nc.declare_dram_parameter("xT", [D, S], BF16, isOutput=False)
    wq = nc.declare_dram_parameter("wq", [D, HPC * HD], BF16, isOutput=False)
    wk = nc.declare_dram_parameter("wk", [D, KVPC * HD], BF16, isOutput=False)
    wv = nc.declare_dram_parameter("wv", [D, KVPC * HD], BF16, isOutput=False)
    wo = nc.declare_dram_parameter("wo", [D, NSLICE], BF16, isOutput=False)
    cos2 = nc.declare_dram_parameter("cos2", [128, S], F32, isOutput=False)
    sin2 = nc.declare_dram_parameter("sin2", [128, S], F32, isOutput=False)
    if mask_mode != "ones":
        maskT = nc.declare_dram_parameter("maskT", [S, S], BF16, isOutput=False)
    out = nc.declare_dram_parameter("out", [S, NSLICE], F32, isOutput=True)

    def nproc_of(qc):
        return 4 * (qc + 1) if mask_mode == "tril" else NKB

    with tile.TileContext(nc) as tc:
        with (
            tc.tile_pool(name="persist", bufs=1) as persist,
            tc.tile_pool(name="dram", bufs=1, space="DRAM") as dram,
        ):
            # persistent SBUF tensors
            qT = persist.tile([128, HPC, S], BF16)  # de-interleaved roped q^T
            kT = persist.tile([128, KVPC, S], BF16)
            V = persist.tile([128, NKB, KVPC * HD], BF16)  # [seq128, blk, kv*hd]
            ones_sb = persist.tile([128, 128], BF16)
            nc.vector.memset(ones_sb[:], 1.0)
            corr = persist.tile([128, KVPC, 4], F32)  # suffix V sums per (kv, qc)

            bounce_in = dram.tile([16, 128, HPC, 128], BF16)  # [m, p, h, s]
            bounce_out = dram.tile([TPG, 16, 128, HPC, 128], BF16)

            # ---------------- phase A: QKV^T projections + RoPE ----------
            with (
                tc.tile_pool(name="pa_w", bufs=1) as pa_w,
                tc.tile_pool(name="pa_x", bufs=1) as pa_x,
                tc.tile_pool(name="pa_t", bufs=2) as pa_t,
                tc.tile_pool(name="pa_ps", bufs=4, space="PSUM") as pa_ps,
                tc.tile_pool(name="pa_psv", bufs=2, space="PSUM") as pa_psv,
                tc.tile_pool(name="pa_psc", bufs=1, space="PSUM") as pa_psc,
            ):
                wq_sb = pa_w.tile([128, KO, HPC * HD], BF16, tag="wq")
                wk_sb = pa_w.tile([128, KO, KVPC * HD], BF16, tag="wk")
                wv_sb = pa_w.tile([128, KO, KVPC * HD], BF16, tag="wv")
                cos_sb = pa_w.tile([128, S], F32, tag="cos")
                sin_sb = pa_w.tile([128, S], F32, tag="sin")
                nc.sync.dma_start(wq_sb[:], wq.ap().rearrange("(ko p) n -> p ko n", p=128))
                nc.sync.dma_start(wk_sb[:], wk.ap().rearrange("(ko p) n -> p ko n", p=128))
                nc.sync.dma_start(wv_sb[:], wv.ap().rearrange("(ko p) n -> p ko n", p=128))
                nc.sync.dma_start(cos_sb[:], cos2.ap())
                nc.sync.dma_start(sin_sb[:], sin2.ap())

                xT_r = xT.ap().rearrange("(ko p) s -> p ko s", p=128)

                def rope_pair(ps_a, ps_b, dstT, h1, h2, sl):
                    """ps_a=[a_h1; a_h2], ps_b=[b_h1; b_h2] -> de-interleaved dstT."""
                    t1 = pa_t.tile([128, SC], F32, tag="t1")
                    t2 = pa_t.tile([128, SC], F32, tag="t2")
                    ra = pa_t.tile([128, SC], BF16, tag="ra")
                    rb = pa_t.tile([128, SC], BF16, tag="rb")
                    nc.vector.tensor_mul(t1[:], ps_a[:], cos_sb[:, sl])
                    nc.vector.tensor_mul(t2[:], ps_b[:], sin_sb[:, sl])
                    nc.vector.tensor_sub(ra[:], t1[:], t2[:])
                    nc.vector.tensor_mul(t1[:], ps_a[:], sin_sb[:, sl])
                    nc.vector.tensor_mul(t2[:], ps_b[:], cos_sb[:, sl])
                    nc.vector.tensor_add(rb[:], t1[:], t2[:])
                    # repack: head rows 0:64 -> dstT[0:64,h1] etc (partition remap)
                    nc.sync.dma_start(dstT[0:64, h1, sl], ra[0:64, :])
                    nc.sync.dma_start(dstT[0:64, h2, sl], ra[64:128, :])
                    nc.sync.dma_start(dstT[64:128, h1, sl], rb[0:64, :])
                    nc.sync.dma_start(dstT[64:128, h2, sl], rb[64:128, :])

                for sc in range(NSC):
                    sl = slice(sc * SC, (sc + 1) * SC)
                    xs = pa_x.tile([128, KO, SC], BF16, tag="x")
                    nc.sync.dma_start(xs[:], xT_r[:, :, sl])
                    # q pairs
                    for j in range(HPC // 2):
                        ps_a = pa_ps.tile([128, SC], F32, tag="ps")
                        ps_b = pa_ps.tile([128, SC], F32, tag="ps")
                        for ko in range(KO):
                            st, sp = ko == 0, ko == KO - 1
                            nc.tensor.matmul(
                                ps_a[:], lhsT=wq_sb[:, ko, j * 256 : j * 256 + 128],
                                rhs=xs[:, ko, :], start=st, stop=sp)
                            nc.tensor.matmul(
                                ps_b[:], lhsT=wq_sb[:, ko, j * 256 + 128 : j * 256 + 256],
                                rhs=xs[:, ko, :], start=st, stop=sp)
                        rope_pair(ps_a, ps_b, qT, 2 * j, 2 * j + 1, sl)
                    # kv pair (KVPC == 2)
                    ps_a = pa_ps.tile([128, SC], F32, tag="ps")
                    ps_b = pa_ps.tile([128, SC], F32, tag="ps")
                    for ko in range(KO):
                        st, sp = ko == 0, ko == KO - 1
                        nc.tensor.matmul(ps_a[:], lhsT=wk_sb[:, ko, 0:128],
                                         rhs=xs[:, ko, :], start=st, stop=sp)
                        nc.tensor.matmul(ps_b[:], lhsT=wk_sb[:, ko, 128:256],
                                         rhs=xs[:, ko, :], start=st, stop=sp)
                    rope_pair(ps_a, ps_b, kT, 0, 1, sl)
                    # V natural [seq, kv*hd]
                    for sb in range(SC // 128):
                        ps_v = pa_psv.tile([128, KVPC * HD], F32, tag="psv")
                        for ko in range(KO):
                            nc.tensor.matmul(
                                ps_v[:], lhsT=xs[:, ko, sb * 128 : sb * 128 + 128],
                                rhs=wv_sb[:, ko, :], start=(ko == 0), stop=(ko == KO - 1))
                        nc.scalar.copy(V[:, sc * 4 + sb, :], ps_v[:])

                # suffix V sums for skipped (fully-masked) chunks
                if mask_mode == "tril":
                    for hk in range(KVPC):
                        for qc in range(3):
                            kcs = list(range(4 * (qc + 1), NKB))
                            psc = pa_psc.tile([128, 1], F32, tag="psc")
                            for i, kc in enumerate(kcs):
                                nc.tensor.matmul(
                                    psc[:], lhsT=V[:, kc, hk * HD : (hk + 1) * HD],
                                    rhs=ones_sb[:, 0:1],
                                    start=(i == 0), stop=(i == len(kcs) - 1))
                            nc.scalar.copy(corr[:, hk, qc : qc + 1], psc[:])

            # ---------------- phase B: attention ------------------------
            with (
                tc.tile_pool(name="pb_m", bufs=1) as pb_m,
                tc.tile_pool(name="pb_e", bufs=6) as pb_e,
                tc.tile_pool(name="pb_s", bufs=4) as pb_s,
                tc.tile_pool(name="pb_o", bufs=1) as pb_o,
                tc.tile_pool(name="pb_ps_s", bufs=3, space="PSUM") as pb_ps_s,
                tc.tile_pool(name="pb_ps_o", bufs=2, space="PSUM") as pb_ps_o,
                tc.tile_pool(name="pb_ps_r", bufs=2, space="PSUM") as pb_ps_r,
            ):
                outT = pb_o.tile([128, HPC, S], BF16)
                if mask_mode == "tril":
                    mask_sb = pb_m.tile([128, 16, QB], BF16)
                    for qc in range(NQB):
                        for r in range(4):
                            kc = 4 * qc + r
                            nc.sync.dma_start(
                                mask_sb[:, qc * 4 + r, :],
                                maskT.ap()[kc * KB : (kc + 1) * KB,
                                           qc * QB : (qc + 1) * QB])
                elif mask_mode == "full":
                    mask_sb = pb_m.tile([128, NKB, QB], BF16)  # per-qc reload

                for h in range(HPC):
                    hk = h // (HPC // KVPC)
                    for qc in range(NQB):
                        nproc = nproc_of(qc)
                        if mask_mode == "full" and h == 0:
                            pass  # loaded below per (qc) on first head
                        if mask_mode == "full":
                            if h == 0:
                                for kc in range(NKB):
                                    nc.sync.dma_start(
                                        mask_sb[:, kc, :],
                                        maskT.ap()[kc * KB : (kc + 1) * KB,
                                                   qc * QB : (qc + 1) * QB])
                        ps_o = pb_ps_o.tile([128, QB], F32, tag="ps_o")
                        ps_r = pb_ps_r.tile([128, QB], F32, tag="ps_r")
                        q_rhs = qT[:, h, qc * QB : (qc + 1) * QB]
                        for i, kc in enumerate(range(nproc)):
                            ps_s = pb_ps_s.tile([128, QB], F32, tag="ps_s")
                            nc.tensor.matmul(
                                ps_s[:], lhsT=kT[:, hk, kc * KB : (kc + 1) * KB],
                                rhs=q_rhs, start=True, stop=True)
                            ex = pb_e.tile([128, QB], BF16, tag="ex")
                            if mask_mode == "tril" and kc >= 4 * qc:
                                msk = mask_sb[:, qc * 4 + (kc - 4 * qc), :]
                                mskd = pb_s.tile([128, QB], F32, tag="mskd")
                                nc.vector.tensor_mul(mskd[:], ps_s[:], msk)
                                nc.scalar.activation(
                                    ex[:], mskd[:],
                                    mybir.ActivationFunctionType.Exp, scale=SCALE)
                            elif mask_mode == "full":
                                mskd = pb_s.tile([128, QB], F32, tag="mskd")
                                nc.vector.tensor_mul(mskd[:], ps_s[:], mask_sb[:, kc, :])
                                nc.scalar.activation(
                                    ex[:], mskd[:],
                                    mybir.ActivationFunctionType.Exp, scale=SCALE)
                            else:
                                nc.scalar.activation(
                                    ex[:], ps_s[:],
                                    mybir.ActivationFunctionType.Exp, scale=SCALE)
                            st, sp = i == 0, i == nproc - 1
                            nc.tensor.matmul(
                                ps_o[:], lhsT=V[:, kc, hk * HD : (hk + 1) * HD],
                                rhs=ex[:], start=st, stop=sp)
                            nc.tensor.matmul(
                                ps_r[:], lhsT=ones_sb[:], rhs=ex[:], start=st, stop=sp)
                        # epilogue: out = (ps_o + corr) / (ps_r + const)
                        n_skip = NKB - nproc
                        rs = pb_s.tile([128, QB], F32, tag="rs")
                        nc.scalar.activation(
                            rs[:], ps_r[:], mybir.ActivationFunctionType.Copy,
                            bias=float(n_skip * KB), scale=1.0)
                        rcp = pb_s.tile([128, QB], F32, tag="rcp")
                        nc.vector.reciprocal_approx_fast(rcp[:], rs[:])
                        dst = outT[:, h, qc * QB : (qc + 1) * QB]
                        if mask_mode == "tril" and n_skip > 0:
                            nc.vector.scalar_tensor_tensor(
                                dst, ps_o[:], corr[:, hk, qc : qc + 1], rcp[:],
                                op0=mybir.AluOpType.add, op1=mybir.AluOpType.mult)
                        else:
                            nc.vector.tensor_mul(dst, ps_o[:], rcp[:])
                    # ship this head's output to the bounce buffer (blocked)
                    nc.sync.dma_start(
                        bounce_in[:, :, h, :].rearrange("m p s -> p m s"),
                        outT[:, h, :].rearrange("p (m s) -> p m s", s=128))

            # ---------------- AllGather + phase C (wo matmul) ------------
            nc.gpsimd.collective_compute(
                "AllGather", mybir.AluOpType.bypass, replica_groups=GROUPS,
                ins=[bounce_in.opt()], outs=[bounce_out.opt()])

            with (
                tc.tile_pool(name="pc_w", bufs=1) as pc_w,
                tc.tile_pool(name="pc_l", bufs=2) as pc_l,
                tc.tile_pool(name="pc_o", bufs=2) as pc_o,
                tc.tile_pool(name="pc_ps", bufs=4, space="PSUM") as pc_ps,
            ):
                wo_sb = pc_w.tile([128, KO, NSLICE], BF16)
                nc.sync.dma_start(wo_sb[:], wo.ap().rearrange("(ko p) n -> p ko n", p=128))
                for m in range(16):
                    lh = pc_l.tile([128, TPG, HPC, 128], BF16, tag="lh")
                    nc.sync.dma_start(
                        lh[:], bounce_out[:, m, :, :, :].rearrange("g p h s -> p g h s"))
                    ps1 = pc_ps.tile([128, 512], F32, tag="pc")
                    ps2 = pc_ps.tile([128, 512], F32, tag="pc")
                    for ko in range(KO):
                        g, hh = divmod(ko, HPC)
                        st, sp = ko == 0, ko == KO - 1
                        nc.tensor.matmul(ps1[:], lhsT=lh[:, g, hh, :],
                                         rhs=wo_sb[:, ko, 0:512], start=st, stop=sp)
                        nc.tensor.matmul(ps2[:], lhsT=lh[:, g, hh, :],
                                         rhs=wo_sb[:, ko, 512:1024], start=st, stop=sp)
                    ob = pc_o.tile([128, NSLICE], F32, tag="ob")
                    nc.scalar.copy(ob[:, 0:512], ps1[:])
                    nc.scalar.copy(ob[:, 512:1024], ps2[:])
                    nc.sync.dma_start(out.ap()[m * 128 : (m + 1) * 128, :], ob[:])

    nc.compile()
    return nc


def _prep_inputs(x, freqs_cos, freqs_sin, mask, wq, wk, wv, wo, mask_mode):
    bf16 = ml_dtypes.bfloat16
    deint = np.concatenate([np.arange(0, HD, 2), np.arange(1, HD, 2)])
    wq4 = wq.reshape(D, H, HD)
    wk4 = wk.reshape(D, KVH, HD)

    cosT = np.ascontiguousarray(freqs_cos.T).astype(np.float32)  # [64, S]
    sinT = np.ascontiguousarray(freqs_sin.T).astype(np.float32)
    cos2 = np.concatenate([cosT, cosT], axis=0)  # [128, S]
    sin2 = np.concatenate([sinT, sinT], axis=0)
    maskT_np = None
    if mask_mode != "ones":
        maskT_np = np.ascontiguousarray(mask.T).astype(bf16)

    in_maps = []
    for c in range(N_CORES):
        b, g = divmod(c, TPG)
        xT = np.ascontiguousarray(x[b].T).astype(bf16)  # [D, S]
        # q: heads g*8 .. g*8+7, packed in pairs (evens | odds)
        cols = []
        for j in range(HPC // 2):
            h1, h2 = g * HPC + 2 * j, g * HPC + 2 * j + 1
            cols.append(wq4[:, h1, 0::2])
            cols.append(wq4[:, h2, 0::2])
            cols.append(wq4[:, h1, 1::2])
            cols.append(wq4[:, h2, 1::2])
        wq_s = np.concatenate(cols, axis=1).astype(bf16)  # [D, 1024]
        k1, k2 = 2 * g, 2 * g + 1
        wk_s = np.concatenate(
            [wk4[:, k1, 0::2], wk4[:, k2, 0::2], wk4[:, k1, 1::2], wk4[:, k2, 1::2]],
            axis=1).astype(bf16)  # [D, 256]
        wv_s = np.ascontiguousarray(
            wv.reshape(D, KVH, HD)[:, 2 * g : 2 * g + 2, :].reshape(D, KVPC * HD)
        ).astype(bf16)
        wo_s = np.ascontiguousarray(wo[:, g * NSLICE : (g + 1) * NSLICE]).astype(bf16)
        m = {
            "xT": xT, "wq": wq_s, "wk": wk_s, "wv": wv_s, "wo": wo_s,
            "cos2": cos2, "sin2": sin2,
        }
        if maskT_np is not None:
            m["maskT"] = maskT_np
        in_maps.append(m)
    return in_maps


def bench(nc, in_maps, iters=12):
    """Wall-clock the compiled graph with device-resident inputs.

    Queues `iters` executions back-to-back (async dispatch amortizes the
    axon round-trip) and reports per-iteration time.
    """
    import time

    import jax
    import jax.numpy as jnp
    from jax.sharding import Mesh, PartitionSpec
    from jax.experimental.shard_map import shard_map
    from concourse import bass2jax
    from concourse.bass2jax import _bass_exec_p, partition_id_tensor

    bass2jax.install_neuronx_cc_hook()
    partition_name = nc.partition_id_tensor.name if nc.partition_id_tensor else None
    in_names, out_names, out_avals, zero_outs = [], [], [], []
    for alloc in nc.m.functions[0].allocations:
        if not isinstance(alloc, mybir.MemoryLocationSet):
            continue
        name = alloc.memorylocations[0].name
        if alloc.kind == "ExternalInput":
            if name != partition_name:
                in_names.append(name)
        elif alloc.kind == "ExternalOutput":
            out_names.append(name)
            shape = tuple(alloc.tensor_shape)
            dtype = mybir.dt.np(alloc.dtype)
            out_avals.append(jax.core.ShapedArray(shape, dtype))
            zero_outs.append(np.zeros(shape, dtype))
    n_params = len(in_names)
    n_outs = len(out_avals)
    all_in_names = list(in_names) + out_names
    if partition_name is not None:
        all_in_names.append(partition_name)

    def _body(*args):
        operands = list(args)
        if partition_name is not None:
            operands.append(partition_id_tensor())
        return tuple(_bass_exec_p.bind(
            *operands, out_avals=tuple(out_avals), in_names=tuple(all_in_names),
            out_names=tuple(out_names), lowering_input_output_aliases=(),
            sim_require_finite=False, sim_require_nnan=False, nc=nc))

    devices = jax.devices()[:N_CORES]
    mesh = Mesh(np.asarray(devices), ("core",))
    in_specs = (PartitionSpec("core"),) * (n_params + n_outs)
    out_specs = (PartitionSpec("core"),) * n_outs
    donate = tuple(range(n_params, n_params + n_outs))
    sharded = jax.jit(
        shard_map(_body, mesh=mesh, in_specs=in_specs, out_specs=out_specs,
                  check_rep=False),
        donate_argnums=donate, keep_unused=True)

    concat_in = [
        jax.device_put(
            np.concatenate([np.asarray(in_maps[c][k]) for c in range(N_CORES)], axis=0))
        for k in in_names
    ]
    mk_zeros = lambda: [
        jax.device_put(np.zeros((N_CORES * z.shape[0], *z.shape[1:]), z.dtype))
        for z in zero_outs
    ]
    # warmup (compiles)
    outs = sharded(*concat_in, *mk_zeros())
    jax.block_until_ready(outs)
    zs = [mk_zeros() for _ in range(iters)]
    t0 = time.perf_counter()
    rs = [sharded(*concat_in, *z) for z in zs]
    jax.block_until_ready(rs)
    t1 = time.perf_counter()
    return (t1 - t0) / iters * 1e9  # ns per iteration


def kernel(x, freqs_cos, freqs_sin, mask, wq, wk, wv, wo, cache_k, cache_v, start_pos):
    global LAST_EXEC_NS
    x = np.asarray(x, np.float32)
    mask = np.asarray(mask, np.float32)
    assert int(start_pos) == 0, "kernel specialized for start_pos == 0"
    assert x.shape == (B, S, D)

    tril = np.tril(np.ones((S, S), np.float32))
    if np.array_equal(mask, tril):
        mask_mode = "tril"
    elif np.all(mask == 1.0):
        mask_mode = "ones"
    else:
        mask_mode = "full"

    if mask_mode not in _CACHE:
        _CACHE[mask_mode] = _build(mask_mode)
    nc = _CACHE[mask_mode]

    in_maps = _prep_inputs(
        x, np.asarray(freqs_cos, np.float32), np.asarray(freqs_sin, np.float32),
        mask, np.asarray(wq, np.float32), np.asarray(wk, np.float32),
        np.asarray(wv, np.float32), np.asarray(wo, np.float32), mask_mode)

    import os
    trace = bool(os.environ.get("KERNEL_TRACE"))
    res = run_bass_kernel_spmd(nc, in_maps, list(range(N_CORES)), trace=trace)
    LAST_EXEC_NS = res.exec_time_ns

    out = np.empty((B, S, D), np.float32)
    for c in range(N_CORES):
        b, g = divmod(c, TPG)
        out[b, :, g * NSLICE : (g + 1) * NSLICE] = res.results[c]["out"]
    return out


# revision 6
# speedup vs baseline: 5.1314x; 5.1314x over previous
"""Distributed Trainium2 kernel for GQA attention block (dense_transformer).

Sharding: DP2 over batch x TP4 over heads (8 cores).
  core c -> batch b = c//4, head-group g = c%4 (8 q heads, 2 kv heads).
Per-core pipeline (one SPMD graph, bf16 matmuls, f32 accumulation):
  phase A: transposed QKV projections (qT/kT: [hd, seq] layout) + fused RoPE
  phase B: scores^T = K_r^T-chunk @ q_r (per head), masked exp via ACT,
           attnV + replicated-ones rowsum matmuls, normalize epilogue
  AllGather (4-core groups) of per-head attention outputs (bf16, blocked)
  phase C: out slice = attn_full @ wo[:, g*1024:(g+1)*1024]
Causal (tril) mask specialization: fully-masked score blocks are skipped;
their exp(0)=1 contributions are added analytically (suffix V sums + counts).
"""

import sys

sys.path.insert(0, "/opt/trn_rl_repo")

import numpy as np
import ml_dtypes

import concourse.bass as bass
import concourse.mybir as mybir
import concourse.tile as tile
from concourse import bacc
from concourse.bass_utils import run_bass_kernel_spmd

BF16 = mybir.dt.bfloat16
F32 = mybir.dt.float32

B, S, D = 2, 2048, 4096
H, KVH, HD = 32, 8, 128
HALF = HD // 2
N_CORES = 8
TPG = 4  # tensor-parallel group size
HPC = H // TPG  # 8 q heads / core
KVPC = KVH // TPG  # 2 kv heads / core
NSLICE = D // TPG  # 1024 output columns / core
SC = 512  # seq chunk (phase A)
NSC = S // SC  # 4
KO = D // 128  # 32 contraction chunks
QB = 512  # q chunk (phase B)
NQB = S // QB  # 4
KB = 128  # k chunk (phase B)
NKB = S // KB  # 16
SCALE = 1.0 / float(np.sqrt(HD))
GROUPS = [[0, 1, 2, 3], [4, 5, 6, 7]]

LAST_EXEC_NS = None
_CACHE = {}


def _build(mask_mode):
    """mask_mode: 'tril' (causal specialization), 'ones' (no mask), 'full'."""
    nc = bacc.Bacc("TRN2", target_bir_lowering=False, debug=False, num_devices=N_CORES)

    xT = nc.declare_dram_parameter("xT", [D, S], BF16, isOutput=False)
    wq = nc.declare_dram_parameter("wq", [D, HPC * HD], BF16, isOutput=False)
    wk = nc.declare_dram_parameter("wk", [D, KVPC * HD], BF16, isOutput=False)
    wv = nc.declare_dram_parameter("wv", [D, KVPC * HD], BF16, isOutput=False)
    wo = nc.declare_dram_parameter("wo", [D, NSLICE], BF16, isOutput=False)
    cos2 = nc.declare_dram_parameter("cos2", [128, S], F32, isOutput=False)
    sin2 = nc.declare_dram_parameter("sin2", [128, S], F32, isOutput=False)
    if mask_mode != "ones":
        maskT = nc.declare_dram_parameter("maskT", [S, S], BF16, isOutput=False)
    out = nc.declare_dram_parameter("out", [S, NSLICE], F32, isOutput=True)

    def nproc_of(qc):
        return 4 * (qc + 1) if mask_mode == "tril" else NKB

    with tile.TileContext(nc) as tc:
        with (
            tc.tile_pool(name="persist", bufs=1) as persist,
            tc.tile_pool(name="dram", bufs=1, space="DRAM") as dram,
        ):
            # persistent SBUF tensors
            qT = persist.tile([128, HPC, S], BF16)  # de-interleaved roped q^T
            kT = persist.tile([128, KVPC, S], BF16)
            V = persist.tile([128, NKB, KVPC * HD], BF16)  # [seq128, blk, kv*hd]
            ones_sb = persist.tile([128, 128], BF16)
            nc.vector.memset(ones_sb[:], 1.0)
            corr = persist.tile([128, KVPC, 4], F32)  # suffix V sums per (kv, qc)

            bounce_in = dram.tile([16, 128, HPC, 128], BF16)  # [m, p, h, s]
            bounce_out = dram.tile([TPG, 16, 128, HPC, 128], BF16)

            # ---------------- phase A: QKV^T projections + RoPE ----------
            with (
                tc.tile_pool(name="pa_w", bufs=1) as pa_w,
                tc.tile_pool(name="pa_x", bufs=1) as pa_x,
                tc.tile_pool(name="pa_t", bufs=2) as pa_t,
                tc.tile_pool(name="pa_ps", bufs=4, space="PSUM") as pa_ps,
                tc.tile_pool(name="pa_psv", bufs=2, space="PSUM") as pa_psv,
                tc.tile_pool(name="pa_psc", bufs=1, space="PSUM") as pa_psc,
            ):
                wq_sb = pa_w.tile([128, KO, HPC * HD], BF16, tag="wq")
                wk_sb = pa_w.tile([128, KO, KVPC * HD], BF16, tag="wk")
                wv_sb = pa_w.tile([128, KO, KVPC * HD], BF16, tag="wv")
                cos_sb = pa_w.tile([128, S], F32, tag="cos")
                sin_sb = pa_w.tile([128, S], F32, tag="sin")
                nc.sync.dma_start(wq_sb[:], wq.ap().rearrange("(ko p) n -> p ko n", p=128))
                nc.sync.dma_start(wk_sb[:], wk.ap().rearrange("(ko p) n -> p ko n", p=128))
                nc.sync.dma_start(wv_sb[:], wv.ap().rearrange("(ko p) n -> p ko n", p=128))
                nc.sync.dma_start(cos_sb[:], cos2.ap())
                nc.sync.dma_start(sin_sb[:], sin2.ap())

                xT_r = xT.ap().rearrange("(ko p) s -> p ko s", p=128)

                def rope_pair(ps_a, ps_b, dstT, h1, h2, sl):
                    """ps_a=[a_h1; a_h2], ps_b=[b_h1; b_h2] -> de-interleaved dstT."""
                    t1 = pa_t.tile([128, SC], F32, tag="t1")
                    t2 = pa_t.tile([128, SC], F32, tag="t2")
                    ra = pa_t.tile([128, SC], BF16, tag="ra")
                    rb = pa_t.tile([128, SC], BF16, tag="rb")
                    nc.vector.tensor_mul(t1[:], ps_a[:], cos_sb[:, sl])
                    nc.vector.tensor_mul(t2[:], ps_b[:], sin_sb[:, sl])
                    nc.vector.tensor_sub(ra[:], t1[:], t2[:])
                    nc.vector.tensor_mul(t1[:], ps_a[:], sin_sb[:, sl])
                    nc.vector.tensor_mul(t2[:], ps_b[:], cos_sb[:, sl])
                    nc.vector.tensor_add(rb[:], t1[:], t2[:])
                    # repack: head rows 0:64 -> dstT[0:64,h1] etc (partition remap)
                    nc.sync.dma_start(dstT[0:64, h1, sl], ra[0:64, :])
                    nc.sync.dma_start(dstT[0:64, h2, sl], ra[64:128, :])
                    nc.sync.dma_start(dstT[64:128, h1, sl], rb[0:64, :])
                    nc.sync.dma_start(dstT[64:128, h2, sl], rb[64:128, :])

                for sc in range(NSC):
                    sl = slice(sc * SC, (sc + 1) * SC)
                    xs = pa_x.tile([128, KO, SC], BF16, tag="x")
                    nc.sync.dma_start(xs[:], xT_r[:, :, sl])
                    # q pairs
                    for j in range(HPC // 2):
                        ps_a = pa_ps.tile([128, SC], F32, tag="ps")
                        ps_b = pa_ps.tile([128, SC], F32, tag="ps")
                        for ko in range(KO):
                            st, sp = ko == 0, ko == KO - 1
                            nc.tensor.matmul(
                                ps_a[:], lhsT=wq_sb[:, ko, j * 256 : j * 256 + 128],
                                rhs=xs[:, ko, :], start=st, stop=sp)
                            nc.tensor.matmul(
                                ps_b[:], lhsT=wq_sb[:, ko, j * 256 + 128 : j * 256 + 256],
                                rhs=xs[:, ko, :], start=st, stop=sp)
                        rope_pair(ps_a, ps_b, qT, 2 * j, 2 * j + 1, sl)
                    # kv pair (KVPC == 2)
                    ps_a = pa_ps.tile([128, SC], F32, tag="ps")
                    ps_b = pa_ps.tile([128, SC], F32, tag="ps")
                    for ko in range(KO):
                        st, sp = ko == 0, ko == KO - 1
                        nc.tensor.matmul(ps_a[:], lhsT=wk_sb[:, ko, 0:128],
                                         rhs=xs[:, ko, :], start=st, stop=sp)
                        nc.tensor.matmul(ps_b[:], lhsT=wk_sb[:, ko, 128:256],
                                         rhs=xs[:, ko, :], start=st, stop=sp)
                    rope_pair(ps_a, ps_b, kT, 0, 1, sl)
                    # V natural [seq, kv*hd]
                    for sb in range(SC // 128):
                        ps_v = pa_psv.tile([128, KVPC * HD], F32, tag="psv")
                        for ko in range(KO):
                            nc.tensor.matmul(
                                ps_v[:], lhsT=xs[:, ko, sb * 128 : sb * 128 + 128],
                                rhs=wv_sb[:, ko, :], start=(ko == 0), stop=(ko == KO - 1))
                        nc.scalar.copy(V[:, sc * 4 + sb, :], ps_v[:])

                # suffix V sums for skipped (fully-masked) chunks
                if mask_mode == "tril":
                    for hk in range(KVPC):
                        for qc in range(3):
                            kcs = list(range(4 * (qc + 1), NKB))
                            psc = pa_psc.tile([128, 1], F32, tag="psc")
                            for i, kc in enumerate(kcs):
                                nc.tensor.matmul(
                                    psc[:], lhsT=V[:, kc, hk * HD : (hk + 1) * HD],
                                    rhs=ones_sb[:, 0:1],
                                    start=(i == 0), stop=(i == len(kcs) - 1))
                            nc.scalar.copy(corr[:, hk, qc : qc + 1], psc[:])

            # ---------------- phase B: attention ------------------------
            with (
                tc.tile_pool(name="pb_m", bufs=1) as pb_m,
                tc.tile_pool(name="pb_e", bufs=6) as pb_e,
                tc.tile_pool(name="pb_s", bufs=4) as pb_s,
                tc.tile_pool(name="pb_o", bufs=1) as pb_o,
                tc.tile_pool(name="pb_ps_s", bufs=3, space="PSUM") as pb_ps_s,
                tc.tile_pool(name="pb_ps_o", bufs=2, space="PSUM") as pb_ps_o,
                tc.tile_pool(name="pb_ps_r", bufs=2, space="PSUM") as pb_ps_r,
            ):
                outT = pb_o.tile([128, HPC, S], BF16)
                if mask_mode == "tril":
                    mask_sb = pb_m.tile([128, 16, QB], BF16)
                    for qc in range(NQB):
                        for r in range(4):
                            kc = 4 * qc + r
                            nc.sync.dma_start(
                                mask_sb[:, qc * 4 + r, :],
                                maskT.ap()[kc * KB : (kc + 1) * KB,
                                           qc * QB : (qc + 1) * QB])
                elif mask_mode == "full":
                    mask_sb = pb_m.tile([128, NKB, QB], BF16)  # per-qc reload

                for h in range(HPC):
                    hk = h // (HPC // KVPC)
                    for qc in range(NQB):
                        nproc = nproc_of(qc)
                        if mask_mode == "full" and h == 0:
                            pass  # loaded below per (qc) on first head
                        if mask_mode == "full":
                            if h == 0:
                                for kc in range(NKB):
                                    nc.sync.dma_start(
                                        mask_sb[:, kc, :],
                                        maskT.ap()[kc * KB : (kc + 1) * KB,
                                                   qc * QB : (qc + 1) * QB])
                        ps_o = pb_ps_o.tile([128, QB], F32, tag="ps_o")
                        ps_r = pb_ps_r.tile([128, QB], F32, tag="ps_r")
                        q_rhs = qT[:, h, qc * QB : (qc + 1) * QB]
                        for i, kc in enumerate(range(nproc)):
                            ps_s = pb_ps_s.tile([128, QB], F32, tag="ps_s")
                            nc.tensor.matmul(
                                ps_s[:], lhsT=kT[:, hk, kc * KB : (kc + 1) * KB],
                                rhs=q_rhs, start=True, stop=True)
                            ex = pb_e.tile([128, QB], BF16, tag="ex")
                            if mask_mode == "tril" and kc >= 4 * qc:
                                msk = mask_sb[:, qc * 4 + (kc - 4 * qc), :]
                                mskd = pb_s.tile([128, QB], F32, tag="mskd")
                                nc.vector.tensor_mul(mskd[:], ps_s[:], msk)
                                nc.scalar.activation(
                                    ex[:], mskd[:],
                                    mybir.ActivationFunctionType.Exp, scale=SCALE)
                            elif mask_mode == "full":
                                mskd = pb_s.tile([128, QB], F32, tag="mskd")
                                nc.vector.tensor_mul(mskd[:], ps_s[:], mask_sb[:, kc, :])
                                nc.scalar.activation(
                                    ex[:], mskd[:],
                                    mybir.ActivationFunctionType.Exp, scale=SCALE)
                            else:
                                nc.scalar.activation(
                                    ex[:], ps_s[:],
                                    mybir.ActivationFunctionType.Exp, scale=SCALE)
                            st, sp = i == 0, i == nproc - 1
                            nc.tensor.matmul(
                                ps_o[:], lhsT=V[:, kc, hk * HD : (hk + 1) * HD],
                                rhs=ex[:], start=st, stop=sp)
                            nc.tensor.matmul(
                                ps_r[:], lhsT=ones_sb[:], rhs=ex[:], start=st, stop=sp)
                        # epilogue: out = (ps_o + corr) / (ps_r + const)
                        n_skip = NKB - nproc
                        rs = pb_s.tile([128, QB], F32, tag="rs")
                        nc.scalar.activation(
                            rs[:], ps_r[:], mybir.ActivationFunctionType.Copy,
                            bias=float(n_skip * KB), scale=1.0)
                        rcp = pb_s.tile([128, QB], F32, tag="rcp")
                        nc.vector.reciprocal_approx_fast(rcp[:], rs[:])
                        dst = outT[:, h, qc * QB : (qc + 1) * QB]
                        if mask_mode == "tril" and n_skip > 0:
                            nc.vector.scalar_tensor_tensor(
                                dst, ps_o[:], corr[:, hk, qc : qc + 1], rcp[:],
                                op0=mybir.AluOpType.add, op1=mybir.AluOpType.mult)
                        else:
                            nc.vector.tensor_mul(dst, ps_o[:], rcp[:])
                    # ship this head's output to the bounce buffer (blocked)
                    nc.sync.dma_start(
                        bounce_in[:, :, h, :].rearrange("m p s -> p m s"),
                        outT[:, h, :].rearrange("p (m s) -> p m s", s=128))

            # ---------------- AllGather + phase C (wo matmul) ------------
            nc.gpsimd.collective_compute(
                "AllGather", mybir.AluOpType.bypass, replica_groups=GROUPS,
                ins=[bounce_in.opt()], outs=[bounce_out.opt()])

            with (
                tc.tile_pool(name="pc_w", bufs=1) as pc_w,
                tc.tile_pool(name="pc_l", bufs=2) as pc_l,
                tc.tile_pool(name="pc_o", bufs=2) as pc_o,
                tc.tile_pool(name="pc_ps", bufs=4, space="PSUM") as pc_ps,
            ):
                wo_sb = pc_w.tile([128, KO, NSLICE], BF16)
                nc.sync.dma_start(wo_sb[:], wo.ap().rearrange("(ko p) n -> p ko n", p=128))
                for m in range(16):
                    lh = pc_l.tile([128, TPG, HPC, 128], BF16, tag="lh")
                    nc.sync.dma_start(
                        lh[:], bounce_out[:, m, :, :, :].rearrange("g p h s -> p g h s"))
                    ps1 = pc_ps.tile([128, 512], F32, tag="pc")
                    ps2 = pc_ps.tile([128, 512], F32, tag="pc")
                    for ko in range(KO):
                        g, hh = divmod(ko, HPC)
                        st, sp = ko == 0, ko == KO - 1
                        nc.tensor.matmul(ps1[:], lhsT=lh[:, g, hh, :],
                                         rhs=wo_sb[:, ko, 0:512], start=st, stop=sp)
                        nc.tensor.matmul(ps2[:], lhsT=lh[:, g, hh, :],
                                         rhs=wo_sb[:, ko, 512:1024], start=st, stop=sp)
                    ob = pc_o.tile([128, NSLICE], F32, tag="ob")
                    nc.scalar.copy(ob[:, 0:512], ps1[:])
                    nc.scalar.copy(ob[:, 512:1024], ps2[:])
                    nc.sync.dma_start(out.ap()[m * 128 : (m + 1) * 128, :], ob[:])

    nc.compile()
    return nc


def _prep_inputs(x, freqs_cos, freqs_sin, mask, wq, wk, wv, wo, mask_mode):
    bf16 = ml_dtypes.bfloat16
    deint = np.concatenate([np.arange(0, HD, 2), np.arange(1, HD, 2)])
    wq4 = wq.reshape(D, H, HD)
    wk4 = wk.reshape(D, KVH, HD)

    cosT = np.ascontiguousarray(freqs_cos.T).astype(np.float32)  # [64, S]
    sinT = np.ascontiguousarray(freqs_sin.T).astype(np.float32)
    cos2 = np.concatenate([cosT, cosT], axis=0)  # [128, S]
    sin2 = np.concatenate([sinT, sinT], axis=0)
    maskT_np = None
    if mask_mode != "ones":
        maskT_np = np.ascontiguousarray(mask.T).astype(bf16)

    in_maps = []
    for c in range(N_CORES):
        b, g = divmod(c, TPG)
        xT = np.ascontiguousarray(x[b].T).astype(bf16)  # [D, S]
        # q: heads g*8 .. g*8+7, packed in pairs (evens | odds)
        cols = []
        for j in range(HPC // 2):
            h1, h2 = g * HPC + 2 * j, g * HPC + 2 * j + 1
            cols.append(wq4[:, h1, 0::2])
            cols.append(wq4[:, h2, 0::2])
            cols.append(wq4[:, h1, 1::2])
            cols.append(wq4[:, h2, 1::2])
        wq_s = np.concatenate(cols, axis=1).astype(bf16)  # [D, 1024]
        k1, k2 = 2 * g, 2 * g + 1
        wk_s = np.concatenate(
            [wk4[:, k1, 0::2], wk4[:, k2, 0::2], wk4[:, k1, 1::2], wk4[:, k2, 1::2]],
            axis=1).astype(bf16)  # [D, 256]
        wv_s = np.ascontiguousarray(
            wv.reshape(D, KVH, HD)[:, 2 * g : 2 * g + 2, :].reshape(D, KVPC * HD)
        ).astype(bf16)
        wo_s = np.ascontiguousarray(wo[:, g * NSLICE : (g + 1) * NSLICE]).astype(bf16)
        m = {
            "xT": xT, "wq": wq_s, "wk": wk_s, "wv": wv_s, "wo": wo_s,
            "cos2": cos2, "sin2": sin2,
        }
        if maskT_np is not None:
            m["maskT"] = maskT_np
        in_maps.append(m)
    return in_maps


def bench(nc, in_maps, iters=12):
    """Wall-clock the compiled graph with device-resident inputs.

    Queues `iters` executions back-to-back (async dispatch amortizes the
    axon round-trip) and reports per-iteration time.
    """
    import time

    import jax
    import jax.numpy as jnp
    from jax.sharding import Mesh, PartitionSpec
    from jax.experimental.shard_map import shard_map
    from concourse import bass2jax
    from concourse.bass2jax import _bass_exec_p, partition_id_tensor

    bass2jax.install_neuronx_cc_hook()
    partition_name = nc.partition_id_tensor.name if nc.partition_id_tensor else None
    in_names, out_names, out_avals, zero_outs = [], [], [], []
    for alloc in nc.m.functions[0].allocations:
        if not isinstance(alloc, mybir.MemoryLocationSet):
            continue
        name = alloc.memorylocations[0].name
        if alloc.kind == "ExternalInput":
            if name != partition_name:
                in_names.append(name)
        elif alloc.kind == "ExternalOutput":
            out_names.append(name)
            shape = tuple(alloc.tensor_shape)
            dtype = mybir.dt.np(alloc.dtype)
            out_avals.append(jax.core.ShapedArray(shape, dtype))
            zero_outs.append(np.zeros(shape, dtype))
    n_params = len(in_names)
    n_outs = len(out_avals)
    all_in_names = list(in_names) + out_names
    if partition_name is not None:
        all_in_names.append(partition_name)

    def _body(*args):
        operands = list(args)
        if partition_name is not None:
            operands.append(partition_id_tensor())
        return tuple(_bass_exec_p.bind(
            *operands, out_avals=tuple(out_avals), in_names=tuple(all_in_names),
            out_names=tuple(out_names), lowering_input_output_aliases=(),
            sim_require_finite=False, sim_require_nnan=False, nc=nc))

    devices = jax.devices()[:N_CORES]
    mesh = Mesh(np.asarray(devices), ("core",))
    in_specs = (PartitionSpec("core"),) * (n_params + n_outs)
    out_specs = (PartitionSpec("core"),) * n_outs
    donate = tuple(range(n_params, n_params + n_outs))
    sharded = jax.jit(
        shard_map(_body, mesh=mesh, in_specs=in_specs, out_specs=out_specs,
                  check_rep=False),
        donate_argnums=donate, keep_unused=True)

    from jax.sharding import NamedSharding

    shard = NamedSharding(mesh, PartitionSpec("core"))
    concat_in = [
        jax.device_put(
            np.concatenate([np.asarray(in_maps[c][k]) for c in range(N_CORES)], axis=0),
            shard)
        for k in in_names
    ]
    mk_zeros = lambda: [
        jax.device_put(np.zeros((N_CORES * z.shape[0], *z.shape[1:]), z.dtype), shard)
        for z in zero_outs
    ]
    # warmup (compiles)
    outs = sharded(*concat_in, *mk_zeros())
    jax.block_until_ready(outs)
    zs = [mk_zeros() for _ in range(iters)]
    t0 = time.perf_counter()
    rs = [sharded(*concat_in, *z) for z in zs]
    jax.block_until_ready(rs)
    t1 = time.perf_counter()
    return (t1 - t0) / iters * 1e9  # ns per iteration


def kernel(x, freqs_cos, freqs_sin, mask, wq, wk, wv, wo, cache_k, cache_v, start_pos):
    global LAST_EXEC_NS
    x = np.asarray(x, np.float32)
    mask = np.asarray(mask, np.float32)
    assert int(start_pos) == 0, "kernel specialized for start_pos == 0"
    assert x.shape == (B, S, D)

    tril = np.tril(np.ones((S, S), np.float32))
    if np.array_equal(mask, tril):
        mask_mode = "tril"
    elif np.all(mask == 1.0):
        mask_mode = "ones"
    else:
        mask_mode = "full"

    if mask_mode not in _CACHE:
        _CACHE[mask_mode] = _build(mask_mode)
    nc = _CACHE[mask_mode]

    in_maps = _prep_inputs(
        x, np.asarray(freqs_cos, np.float32), np.asarray(freqs_sin, np.float32),
        mask, np.asarray(wq, np.float32), np.asarray(wk, np.float32),
        np.asarray(wv, np.float32), np.asarray(wo, np.float32), mask_mode)

    import os
    trace = bool(os.environ.get("KERNEL_TRACE"))
    res = run_bass_kernel_spmd(nc, in_maps, list(range(N_CORES)), trace=trace)
    LAST_EXEC_NS = res.exec_time_ns

    out = np.empty((B, S, D), np.float32)
    for c in range(N_CORES):
        b, g = divmod(c, TPG)
        out[b, :, g * NSLICE : (g + 1) * NSLICE] = res.results[c]["out"]
    return out
